# revision 1
# baseline (speedup 1.0000x reference)
"""Trainium2 Bass kernel for nn_Net_SLSTM_Conv (conv1d -> spiking LSTM -> BN ->
spiking LSTM -> mean -> fc), data-parallel over the T=512 axis on 8 cores.

Layout strategy (per core, T-chunk of 64 columns):
  - Everything feature-major: [features on partitions, t-columns on free dim].
  - Conv1d folded into one K=85 matmul (bf16 hi/lo split of x + ones row for bias).
  - Gate preactivations accumulate in a per-step PSUM bank [128, 4*64]
    (gates ordered i,f,o,g; gate g pre-scaled by 2 so one sigmoid op serves all
    four gates: tanh(x) = 2*sigmoid(2x)-1).
  - Layer-1 bias via a ones-row in the spike input (K=33); layer-2 bias via a
    K=4 selector matmul (bias depends on BN stats, folded on device).
  - BN over (B*T, H) of binary spikes reduces to a spike-count AllReduce;
    normalization folds into layer-2 input weights/bias on device.
  - mean-over-steps + fc fold into a single accumulating K=128->M=8 matmul.
"""
import os
import numpy as np
import ml_dtypes

import concourse.bass as bass
import concourse.mybir as mybir
import concourse.tile as tile
from concourse.bass_utils import run_bass_kernel_spmd

BF = mybir.dt.bfloat16
F32 = mybir.dt.float32
AF = mybir.ActivationFunctionType
OP = mybir.AluOpType

NCORES = 8
B, T, C = 256, 512, 14
H = 128
CH = 32          # conv output channels
TC = T // NCORES  # 64 t-columns per core
STEPS = int(os.environ.get("SLSTM_STEPS", B))  # debug override
EPS = 1e-5
GBUFS = 4        # PSUM step-bank rotation depth


def _bf16(x):
    return np.asarray(x, np.float32).astype(ml_dtypes.bfloat16)


def _reorder_gates_cols(wt):
    # [*, 512] gate-major cols in torch order i,f,g,o -> (g,i,f,o), scale g by 2
    # (g first so the chain-critical sigmoid over g,i,f can issue after 3 MMs)
    i, f, g, o = (wt[..., k * H:(k + 1) * H] for k in range(4))
    return np.concatenate([2.0 * g, i, f, o], axis=-1)


def build_kernel(thr1: float, thr2: float):
    nc = bass.Bass()

    # ---- external I/O ----
    xt3_d = nc.dram_tensor("xt3", [85, B * TC], BF, kind="ExternalInput")
    wconv_d = nc.dram_tensor("wconv", [85, CH], BF, kind="ExternalInput")
    w1t_d = nc.dram_tensor("w1t", [33, 4 * H], BF, kind="ExternalInput")
    whh1t_d = nc.dram_tensor("whh1t", [H, 4 * H], BF, kind="ExternalInput")
    w2t32_d = nc.dram_tensor("w2t32", [H, 4 * H], F32, kind="ExternalInput")
    w2tbf_d = nc.dram_tensor("w2tbf", [H, 4 * H], BF, kind="ExternalInput")
    whh2t_d = nc.dram_tensor("whh2t", [H, 4 * H], BF, kind="ExternalInput")
    b2sum_d = nc.dram_tensor("b2sum", [1, 4 * H], F32, kind="ExternalInput")
    sel4_d = nc.dram_tensor("sel4", [4, 4 * TC], BF, kind="ExternalInput")
    fcwt_d = nc.dram_tensor("fcwt", [H, 8], BF, kind="ExternalInput")
    fcb_d = nc.dram_tensor("fcb", [8, 1], F32, kind="ExternalInput")
    gamma_d = nc.dram_tensor("gamma", [H, 1], F32, kind="ExternalInput")
    beta_d = nc.dram_tensor("beta", [H, 1], F32, kind="ExternalInput")
    out_d = nc.dram_tensor("out", [8, TC], F32, kind="ExternalOutput")
    DBG = bool(int(os.environ.get("SLSTM_DEBUG", "0")))
    if DBG:
        spk0_dd = nc.dram_tensor("spk0_d", [33, B * TC], BF, kind="ExternalOutput")
        spk1_dd = nc.dram_tensor("spk1_d", [H, B * TC], BF, kind="ExternalOutput")
        cnt_dd = nc.dram_tensor("cnt_d", [H, 1], F32, kind="ExternalOutput")
        b2p_dd = nc.dram_tensor("b2p_d", [4, H], BF, kind="ExternalOutput")
        w2e_dd = nc.dram_tensor("w2e_d", [H, 4 * H], BF, kind="ExternalOutput")

    with tile.TileContext(nc) as tc:
        import contextlib
        ctx = contextlib.ExitStack()
        with ctx:
            const = ctx.enter_context(tc.tile_pool(name="const", bufs=1))
            big = ctx.enter_context(tc.tile_pool(name="big", bufs=1))
            spool = ctx.enter_context(tc.tile_pool(name="spool", bufs=3))
            vpool = ctx.enter_context(tc.tile_pool(name="vpool", bufs=3))
            stpool = ctx.enter_context(tc.tile_pool(name="stpool", bufs=2))
            gpool = ctx.enter_context(
                tc.tile_pool(name="gpool", bufs=GBUFS, space="PSUM"))
            cpool = ctx.enter_context(
                tc.tile_pool(name="cpool", bufs=2, space="PSUM"))
            fpool = ctx.enter_context(
                tc.tile_pool(name="fpool", bufs=1, space="PSUM"))
            dram = ctx.enter_context(
                tc.tile_pool(name="dram", bufs=1, space="DRAM"))

            # ---- load constants ----
            def load(pool, dt_, dram_t, shape):
                t_ = pool.tile(shape, dt_, name=dram_t.name + "_sb")
                nc.sync.dma_start(t_[:], dram_t[:])
                return t_

            xt3_sb = load(big, BF, xt3_d, [85, B * TC])
            wconv_sb = load(const, BF, wconv_d, [85, CH])
            w1t_sb = load(const, BF, w1t_d, [33, 4 * H])
            whh1t_sb = load(const, BF, whh1t_d, [H, 4 * H])
            w2t32_sb = load(const, F32, w2t32_d, [H, 4 * H])
            w2tbf_sb = load(const, BF, w2tbf_d, [H, 4 * H])
            whh2t_sb = load(const, BF, whh2t_d, [H, 4 * H])
            b2sum_sb = load(const, F32, b2sum_d, [1, 4 * H])
            sel4_sb = load(const, BF, sel4_d, [4, 4 * TC])
            fcwt_sb = load(const, BF, fcwt_d, [H, 8])
            fcb_sb = load(const, F32, fcb_d, [8, 1])
            gamma_sb = load(const, F32, gamma_d, [H, 1])
            beta_sb = load(const, F32, beta_d, [H, 1])

            spk0_sb = big.tile([33, B * TC], BF, name="spk0")
            spk1_sb = big.tile([H, B * TC], BF, name="spk1")
            zeros_sb = const.tile([H, TC], BF, name="zeros")
            nc.vector.memset(zeros_sb[:], 0.0)
            nc.vector.memset(spk0_sb[32:33, :], 1.0)  # ones row = layer-1 bias path

            # ---- conv + spike threshold ----
            NCHUNK = (B * TC) // 512
            for cchunk in range(NCHUNK):
                cp = cpool.tile([CH, 512], F32, name="convp", tag="convp")
                sl = slice(cchunk * 512, (cchunk + 1) * 512)
                nc.tensor.matmul(cp[:, :], wconv_sb[:, :], xt3_sb[:, sl],
                                 start=True, stop=True)
                nc.vector.tensor_scalar(spk0_sb[0:CH, sl], cp[:, :],
                                        1.0, 0.0, OP.subtract, OP.is_gt)

            # ---- the recurrent step (shared between both layers) ----
            def lstm_step(b, layer, syn_prev, mem_prev, spk_prev_ap, thr):
                gb = gpool.tile([H, 4 * TC], F32, name="gbank", tag="gbank")
                if layer == 1:
                    rhs_in = spk0_sb[:, b * TC:(b + 1) * TC]
                    for g in range(4):
                        nc.tensor.matmul(gb[:, g * TC:(g + 1) * TC],
                                         w1t_sb[:, g * H:(g + 1) * H], rhs_in,
                                         start=(g == 0), stop=False)
                else:
                    # bias selector first: fills the whole bank
                    nc.tensor.matmul(gb[:, :], b2p_sb[:, :], sel4_sb[:, :],
                                     start=True, stop=False)
                    rhs_in = spk1_sb[:, b * TC:(b + 1) * TC]
                    for g in range(4):
                        nc.tensor.matmul(gb[:, g * TC:(g + 1) * TC],
                                         w2eff_sb[:, g * H:(g + 1) * H], rhs_in,
                                         start=False, stop=False)
                whh = whh1t_sb if layer == 1 else whh2t_sb
                for g in range(4):
                    nc.tensor.matmul(gb[:, g * TC:(g + 1) * TC],
                                     whh[:, g * H:(g + 1) * H], mem_prev[:, :],
                                     start=False, stop=(g == 3))
                # gate order in bank: g' | i | f | o
                S = spool.tile([H, 4 * TC], BF, name="S", tag="S")
                nc.scalar.activation(S[:, 0:3 * TC], gb[:, 0:3 * TC], AF.Sigmoid)
                nc.scalar.activation(S[:, 3 * TC:], gb[:, 3 * TC:], AF.Sigmoid)
                u = vpool.tile([H, TC], BF, name="u", tag="u")
                nc.vector.scalar_tensor_tensor(
                    u[:], S[:, 0:TC], 0.5, S[:, TC:2 * TC],
                    op0=OP.subtract, op1=OP.mult)          # (g'-0.5)*i = i*g/2
                fs = vpool.tile([H, TC], BF, name="fs", tag="fs")
                nc.vector.tensor_tensor(fs[:], S[:, 2 * TC:3 * TC], syn_prev[:],
                                        op=OP.mult)
                syn = stpool.tile([H, TC], BF, name="syn", tag="syn")
                nc.vector.scalar_tensor_tensor(
                    syn[:], u[:], 2.0, fs[:], op0=OP.mult, op1=OP.add)
                ts = vpool.tile([H, TC], BF, name="ts", tag="ts")
                nc.scalar.activation(ts[:], syn[:], AF.Tanh)
                mp = vpool.tile([H, TC], BF, name="mp", tag="mp")
                nc.vector.tensor_tensor(mp[:], S[:, 3 * TC:4 * TC], ts[:],
                                        op=OP.mult)
                mem = stpool.tile([H, TC], BF, name="mem", tag="mem")
                nc.vector.scalar_tensor_tensor(
                    mem[:], spk_prev_ap, -thr, mp[:], op0=OP.mult, op1=OP.add)
                return syn, mem

            # ---- phase A: layer-1 scan, record spikes + counts ----
            syn_p, mem_p, spk_p = zeros_sb, zeros_sb, zeros_sb[:]
            for b in range(STEPS):
                syn_p, mem_p = lstm_step(b, 1, syn_p, mem_p, spk_p, thr1)
                spk_slice = spk1_sb[:, b * TC:(b + 1) * TC]
                nc.vector.tensor_scalar(spk_slice, mem_p[:], thr1, 0.0,
                                        OP.subtract, OP.is_gt)
                spk_p = spk_slice

            # ---- BN stats: count -> AllReduce -> fold into layer-2 weights ----
            cnt = const.tile([H, 1], F32, name="cnt")
            nc.vector.tensor_reduce(cnt[:], spk1_sb[:, 0:STEPS * TC],
                                    axis=mybir.AxisListType.X, op=OP.add)
            cc_in = dram.tile([H, 1], F32, name="cc_in")
            cc_out = dram.tile([H, 1], F32, name="cc_out", addr_space="Shared")
            nc.sync.dma_start(cc_in[:], cnt[:])
            nc.gpsimd.collective_compute(
                "AllReduce", OP.add,
                replica_groups=[list(range(NCORES))],
                ins=[cc_in[:]], outs=[cc_out[:]])
            cntg = const.tile([H, 1], F32, name="cntg")
            nc.sync.dma_start(cntg[:], cc_out[:])

            p_t = const.tile([H, 1], F32, name="p_t")
            nc.vector.tensor_scalar(p_t[:], cntg[:], 1.0 / (B * T), None, OP.mult)
            q_t = const.tile([H, 1], F32, name="q_t")
            nc.vector.tensor_scalar(q_t[:], p_t[:], -1.0, 1.0, OP.mult, OP.add)
            var_t = const.tile([H, 1], F32, name="var_t")
            nc.vector.tensor_tensor(var_t[:], p_t[:], q_t[:], op=OP.mult)
            nc.vector.tensor_scalar(var_t[:], var_t[:], EPS, None, OP.add)
            sq_t = const.tile([H, 1], F32, name="sq_t")
            nc.scalar.activation(sq_t[:], var_t[:], AF.Sqrt, bias=0.0)
            rs_t = const.tile([H, 1], F32, name="rs_t")
            nc.vector.reciprocal(rs_t[:], sq_t[:])
            a_t = const.tile([H, 1], F32, name="a_t")
            nc.vector.tensor_tensor(a_t[:], gamma_sb[:], rs_t[:], op=OP.mult)
            pa_t = const.tile([H, 1], F32, name="pa_t")
            nc.vector.tensor_tensor(pa_t[:], p_t[:], a_t[:], op=OP.mult)
            c_t = const.tile([H, 1], F32, name="c_t")
            nc.vector.scalar_tensor_tensor(c_t[:], pa_t[:], -1.0, beta_sb[:],
                                           op0=OP.mult, op1=OP.add)
            cbf_t = const.tile([H, 1], BF, name="cbf_t")
            nc.vector.tensor_copy(cbf_t[:], c_t[:])

            w2eff_sb = const.tile([H, 4 * H], BF, name="w2eff")
            nc.vector.tensor_scalar(w2eff_sb[:], w2t32_sb[:], a_t[:], None, OP.mult)

            bp = fpool.tile([1, 4 * H], F32, name="biasp", tag="biasp")
            nc.tensor.matmul(bp[:, :], cbf_t[:, :], w2tbf_sb[:, :],
                             start=True, stop=True)
            b2eff_sb = const.tile([1, 4 * H], BF, name="b2eff")
            nc.vector.tensor_tensor(b2eff_sb[:], b2sum_sb[:], bp[:, :], op=OP.add)
            # reshape [1,512] -> [4,128] across partitions via linear DRAM
            b2lin = dram.tile([4, H], BF, name="b2lin")
            nc.sync.dma_start(b2lin[:].rearrange("a b -> () (a b)"), b2eff_sb[:])
            b2p_sb = const.tile([4, H], BF, name="b2p")
            nc.sync.dma_start(b2p_sb[:], b2lin[:])

            # ---- phase B: layer-2 scan, fused mean+fc accumulation ----
            fcp = fpool.tile([8, TC], F32, name="fcp", tag="fcp")
            syn_p, mem_p = zeros_sb, zeros_sb
            spk_p = zeros_sb[:]
            for b in range(STEPS):
                syn_p, mem_p = lstm_step(b, 2, syn_p, mem_p, spk_p, thr2)
                spk_n = stpool.tile([H, TC], BF, name="spk2", tag="spk2")
                nc.vector.tensor_scalar(spk_n[:], mem_p[:], thr2, 0.0,
                                        OP.subtract, OP.is_gt)
                spk_p = spk_n[:]
                nc.tensor.matmul(fcp[:, :], fcwt_sb[:, :], mem_p[:, :],
                                 start=(b == 0), stop=(b == STEPS - 1))

            out_sb = const.tile([8, TC], F32, name="out_sb")
            nc.scalar.activation(out_sb[:], fcp[:, :], AF.Identity,
                                 bias=fcb_sb[:])
            nc.sync.dma_start(out_d[:], out_sb[:])

            if DBG:
                nc.sync.dma_start(spk0_dd[:], spk0_sb[:])
                nc.sync.dma_start(spk1_dd[:], spk1_sb[:])
                nc.sync.dma_start(cnt_dd[:], cnt[:])
                nc.sync.dma_start(b2p_dd[:], b2p_sb[:])
                nc.sync.dma_start(w2e_dd[:], w2eff_sb[:])

    _split_mm_waits(nc)
    return nc


def _split_mm_waits(nc):
    """The S3D3 matmul ISA struct carries only one sync-wait slot; move any
    extra Tile-assigned waits onto a preceding PE NoOp."""
    for fn in nc.m.functions:
        for blk in fn.blocks:
            out = []
            for inst in blk.instructions:
                si = getattr(inst, "sync_info", None)
                if (not isinstance(inst, (mybir.InstEventSemaphore,
                                          mybir.InstAllEngineBarrier))
                        and si is not None and si.on_wait
                        and len(si.on_wait) > 1):
                    for j, w in enumerate(si.on_wait[:-1]):
                        nop = mybir.InstNoOp(name=f"{inst.name}-wsplit{j}",
                                             ins=[], outs=[])
                        nop.engine = inst.engine
                        nop.sync_info = mybir.SyncInfo(on_wait=[w],
                                                       on_update=[])
                        out.append(nop)
                    si.on_wait = [si.on_wait[-1]]
                out.append(inst)
            blk.instructions[:] = out


def _host_inputs(x, conv_w, conv_b, w_ih1, w_hh1, b_ih1, b_hh1,
                 w_ih2, w_hh2, b_ih2, b_hh2, bn_gamma, bn_beta, fc_w, fc_b):
    """Build the per-core input dicts (numpy, host-side)."""
    f32 = np.float32
    # im2col with hi/lo bf16 split per core
    xp = np.pad(np.asarray(x, f32), ((0, 0), (1, 1), (0, 0)))  # [B, T+2, C]
    common = {}
    w3t = np.concatenate([conv_w[:, :, k].T for k in range(3)], axis=0)  # [42,32]
    common["wconv"] = _bf16(np.concatenate(
        [w3t, w3t, np.asarray(conv_b, f32)[None, :]], axis=0))
    w1t = _reorder_gates_cols(np.asarray(w_ih1, f32).T)        # [32, 512]
    b1 = _reorder_gates_cols((np.asarray(b_ih1) + np.asarray(b_hh1))[None, :])
    common["w1t"] = _bf16(np.concatenate([w1t, b1], axis=0))   # [33, 512]
    common["whh1t"] = _bf16(_reorder_gates_cols(np.asarray(w_hh1, f32).T))
    w2t = _reorder_gates_cols(np.asarray(w_ih2, f32).T)        # [128, 512]
    common["w2t32"] = np.ascontiguousarray(w2t, f32)
    common["w2tbf"] = _bf16(w2t)
    common["whh2t"] = _bf16(_reorder_gates_cols(np.asarray(w_hh2, f32).T))
    common["b2sum"] = np.ascontiguousarray(
        _reorder_gates_cols((np.asarray(b_ih2) + np.asarray(b_hh2))[None, :]), f32)
    sel = np.zeros((4, 4 * TC), f32)
    for g in range(4):
        sel[g, g * TC:(g + 1) * TC] = 1.0
    common["sel4"] = _bf16(sel)
    common["fcwt"] = _bf16(np.asarray(fc_w, f32).T / STEPS)
    common["fcb"] = np.ascontiguousarray(np.asarray(fc_b, f32)[:, None], f32)
    common["gamma"] = np.ascontiguousarray(np.asarray(bn_gamma, f32)[:, None], f32)
    common["beta"] = np.ascontiguousarray(np.asarray(bn_beta, f32)[:, None], f32)

    in_maps = []
    for k in range(NCORES):
        xw = xp[:, 64 * k: 64 * k + 66, :]                     # [B, 66, C]
        taps = [xw[:, kk:kk + 64, :].transpose(2, 0, 1).reshape(C, B * TC)
                for kk in range(3)]                            # 3 x [14, B*64]
        arr = np.concatenate(taps, axis=0)                     # [42, B*64]
        hi = arr.astype(ml_dtypes.bfloat16)
        lo = (arr - hi.astype(f32)).astype(ml_dtypes.bfloat16)
        ones = np.ones((1, B * TC), ml_dtypes.bfloat16)
        m = dict(common)
        m["xt3"] = np.ascontiguousarray(np.concatenate(
            [hi, lo, ones], axis=0))                           # [85, B*64]
        in_maps.append(m)
    return in_maps


_CACHE = {}


def kernel(x, conv_w, conv_b, w_ih1, w_hh1, b_ih1, b_hh1, thr1,
           w_ih2, w_hh2, b_ih2, b_hh2, thr2, bn_gamma, bn_beta,
           fc_w, fc_b):
    thr1 = float(np.asarray(thr1)); thr2 = float(np.asarray(thr2))
    key = (thr1, thr2)
    if key not in _CACHE:
        _CACHE[key] = build_kernel(thr1, thr2)
    nc = _CACHE[key]
    in_maps = _host_inputs(x, conv_w, conv_b, w_ih1, w_hh1, b_ih1, b_hh1,
                           w_ih2, w_hh2, b_ih2, b_hh2, bn_gamma, bn_beta,
                           fc_w, fc_b)
    res = run_bass_kernel_spmd(nc, in_maps, core_ids=list(range(NCORES)),
                               trace=bool(int(os.environ.get("SLSTM_TRACE", "0"))))
    outT = np.concatenate([r["out"] for r in res.results], axis=1)  # [8, 512]
    if res.exec_time_ns is not None:
        kernel.last_exec_time_ns = res.exec_time_ns
    return np.ascontiguousarray(outT.T.astype(np.float32))



# revision 4
# speedup vs baseline: 1.1414x; 1.1414x over previous
"""Trainium2 Bass kernel for nn_Net_SLSTM_Conv (conv1d -> spiking LSTM -> BN ->
spiking LSTM -> mean -> fc), data-parallel over the T=512 axis on 8 cores.

Layout strategy (per core, T-chunk of 64 columns):
  - Everything feature-major: [features on partitions, t-columns on free dim].
  - Conv1d folded into one K=85 matmul (bf16 hi/lo split of x + ones row for bias).
  - Gate preactivations accumulate in a per-step PSUM bank [128, 4*64]
    (gates ordered g,i,f,o; gate g pre-scaled by 2 so one sigmoid op serves
    g,i,f: tanh(x) = 2*sigmoid(2x)-1).
  - mem is never materialized: mem(b) = mp(b) - thr*spk(b-1) with
    mp = o*tanh(syn), so the recurrent matmul splits into whh@mp (on the
    critical chain) and -thr*whh@spk (off-chain, spk known a step early).
  - Cell state kept as z = syn/2 so the update is z = u + fs with
    u = (S_g-0.5)*S_i (STT) and fs = S_f*z_prev (2x-mode TT); tanh applies
    the missing 2 via the activation input scale.
  - Spikes via STT (mp - thr) > spk_prev (binary, thr folded), with
    accum_out accumulating per-step spike counts for BN (no big reduce).
  - BN over (B*T, H) of binary spikes reduces to a spike-count AllReduce;
    normalization folds into layer-2 input weights/bias on device.
  - mean-over-steps + fc fold into accumulating K=128->M=8 matmuls over
    mp and spk2.
"""
import os
import numpy as np
import ml_dtypes

import concourse.bass as bass
import concourse.mybir as mybir
import concourse.tile as tile
from concourse.bass_utils import run_bass_kernel_spmd

BF = mybir.dt.bfloat16
F32 = mybir.dt.float32
AF = mybir.ActivationFunctionType
OP = mybir.AluOpType

NCORES = 8
B, T, C = 256, 512, 14
H = 128
CH = 32          # conv output channels
TC = T // NCORES  # 64 t-columns per core
STEPS = int(os.environ.get("SLSTM_STEPS", B))  # debug override
EPS = 1e-5
GBUFS = 4        # PSUM step-bank rotation depth


def _bf16(x):
    return np.asarray(x, np.float32).astype(ml_dtypes.bfloat16)


def _reorder_gates_cols(wt):
    # [*, 512] gate-major cols in torch order i,f,g,o -> (g,i,f,o), scale g by 2
    # (g first so sigmoid(2g) can serve as tanh via 2*sig-1)
    i, f, g, o = (wt[..., k * H:(k + 1) * H] for k in range(4))
    return np.concatenate([2.0 * g, i, f, o], axis=-1)


def build_kernel(thr1: float, thr2: float):
    nc = bass.Bass()

    # ---- external I/O ----
    xt3_d = nc.dram_tensor("xt3", [85, B * TC], BF, kind="ExternalInput")
    wconv_d = nc.dram_tensor("wconv", [85, CH], BF, kind="ExternalInput")
    w1t_d = nc.dram_tensor("w1t", [33, 4 * H], BF, kind="ExternalInput")
    whh1t_d = nc.dram_tensor("whh1t", [H, 4 * H], BF, kind="ExternalInput")
    wspk1t_d = nc.dram_tensor("wspk1t", [H, 4 * H], BF, kind="ExternalInput")
    w2t32_d = nc.dram_tensor("w2t32", [H, 4 * H], F32, kind="ExternalInput")
    w2tbf_d = nc.dram_tensor("w2tbf", [H, 4 * H], BF, kind="ExternalInput")
    whh2t_d = nc.dram_tensor("whh2t", [H, 4 * H], BF, kind="ExternalInput")
    wspk2t_d = nc.dram_tensor("wspk2t", [H, 4 * H], BF, kind="ExternalInput")
    b2sum_d = nc.dram_tensor("b2sum", [1, 4 * H], F32, kind="ExternalInput")
    sel4_d = nc.dram_tensor("sel4", [4, 4 * TC], BF, kind="ExternalInput")
    fcwt_d = nc.dram_tensor("fcwt", [H, 8], BF, kind="ExternalInput")
    fcwtn_d = nc.dram_tensor("fcwtn", [H, 8], BF, kind="ExternalInput")
    fcb_d = nc.dram_tensor("fcb", [8, 1], F32, kind="ExternalInput")
    gamma_d = nc.dram_tensor("gamma", [H, 1], F32, kind="ExternalInput")
    beta_d = nc.dram_tensor("beta", [H, 1], F32, kind="ExternalInput")
    out_d = nc.dram_tensor("out", [8, TC], F32, kind="ExternalOutput")
    DBG = bool(int(os.environ.get("SLSTM_DEBUG", "0")))
    if DBG:
        spk0_dd = nc.dram_tensor("spk0_d", [33, B * TC], BF, kind="ExternalOutput")
        spk1_dd = nc.dram_tensor("spk1_d", [H, B * TC], BF, kind="ExternalOutput")
        cnt_dd = nc.dram_tensor("cnt_d", [H, 1], F32, kind="ExternalOutput")
        b2p_dd = nc.dram_tensor("b2p_d", [4, H], BF, kind="ExternalOutput")
        w2e_dd = nc.dram_tensor("w2e_d", [H, 4 * H], BF, kind="ExternalOutput")

    with tile.TileContext(nc) as tc:
        import contextlib
        ctx = contextlib.ExitStack()
        with ctx:
            const = ctx.enter_context(tc.tile_pool(name="const", bufs=1))
            big = ctx.enter_context(tc.tile_pool(name="big", bufs=1))
            s3pool = ctx.enter_context(tc.tile_pool(name="s3pool", bufs=2))
            sopool = ctx.enter_context(tc.tile_pool(name="sopool", bufs=2))
            vpool = ctx.enter_context(tc.tile_pool(name="vpool", bufs=3))
            zpool = ctx.enter_context(tc.tile_pool(name="zpool", bufs=2))
            mppool = ctx.enter_context(tc.tile_pool(name="mppool", bufs=3))
            skpool = ctx.enter_context(tc.tile_pool(name="skpool", bufs=3))
            gpool = ctx.enter_context(
                tc.tile_pool(name="gpool", bufs=GBUFS, space="PSUM"))
            cpool = ctx.enter_context(
                tc.tile_pool(name="cpool", bufs=2, space="PSUM"))
            fpool = ctx.enter_context(
                tc.tile_pool(name="fpool", bufs=1, space="PSUM"))
            dram = ctx.enter_context(
                tc.tile_pool(name="dram", bufs=1, space="DRAM"))

            # ---- load constants ----
            def load(pool, dt_, dram_t, shape):
                t_ = pool.tile(shape, dt_, name=dram_t.name + "_sb")
                nc.sync.dma_start(t_[:], dram_t[:])
                return t_

            xt3_sb = load(big, BF, xt3_d, [85, B * TC])
            wconv_sb = load(const, BF, wconv_d, [85, CH])
            w1t_sb = load(const, BF, w1t_d, [33, 4 * H])
            whh1t_sb = load(const, BF, whh1t_d, [H, 4 * H])
            wspk1t_sb = load(const, BF, wspk1t_d, [H, 4 * H])
            w2t32_sb = load(const, F32, w2t32_d, [H, 4 * H])
            w2tbf_sb = load(const, BF, w2tbf_d, [H, 4 * H])
            whh2t_sb = load(const, BF, whh2t_d, [H, 4 * H])
            wspk2t_sb = load(const, BF, wspk2t_d, [H, 4 * H])
            b2sum_sb = load(const, F32, b2sum_d, [1, 4 * H])
            sel4_sb = load(const, BF, sel4_d, [4, 4 * TC])
            fcwt_sb = load(const, BF, fcwt_d, [H, 8])
            fcwtn_sb = load(const, BF, fcwtn_d, [H, 8])
            fcb_sb = load(const, F32, fcb_d, [8, 1])
            gamma_sb = load(const, F32, gamma_d, [H, 1])
            beta_sb = load(const, F32, beta_d, [H, 1])

            spk0_sb = big.tile([33, B * TC], BF, name="spk0")
            spk1_sb = big.tile([H, B * TC], BF, name="spk1")
            cntcol = big.tile([H, STEPS], F32, name="cntcol")
            zeros_sb = const.tile([H, TC], BF, name="zeros")
            nc.vector.memset(zeros_sb[:], 0.0)
            nc.vector.memset(spk0_sb[32:33, :], 1.0)  # ones row = layer-1 bias path

            # ---- conv + spike threshold ----
            NCHUNK = (B * TC) // 512
            for cchunk in range(NCHUNK):
                cp = cpool.tile([CH, 512], F32, name="convp", tag="convp")
                sl = slice(cchunk * 512, (cchunk + 1) * 512)
                nc.tensor.matmul(cp[:, :], wconv_sb[:, :], xt3_sb[:, sl],
                                 start=True, stop=True)
                nc.vector.tensor_scalar(spk0_sb[0:CH, sl], cp[:, :],
                                        1.0, 0.0, OP.subtract, OP.is_gt)

            # ---- the recurrent step (shared between both layers) ----
            # state passed between iterations:
            #   z_prev  = syn(b-1)/2          [H, TC] bf16
            #   mp_prev = o*tanh(syn) (b-1)   [H, TC] bf16
            #   sk1     = spk(b-1) access-pattern (rhs for gates(b+1) spk mms,
            #             and in1 of the spike STT)
            #   sk2     = spk(b-2) access-pattern (rhs for gates(b) spk mms)
            def lstm_step(b, layer, z_prev, mp_prev, sk1, sk2, thr):
                gb = gpool.tile([H, 4 * TC], F32, name="gbank", tag="gbank")
                whh = whh1t_sb if layer == 1 else whh2t_sb
                wspk = wspk1t_sb if layer == 1 else wspk2t_sb
                # --- PE: gates(b) = input(b) [+ wspk@spk(b-2)] [+ whh@mp(b-1)]
                in_stop = b == 0  # no mp mms at b=0: input group closes the bank
                if layer == 1:
                    rhs_in = spk0_sb[:, b * TC:(b + 1) * TC]
                    for g in range(4):
                        nc.tensor.matmul(gb[:, g * TC:(g + 1) * TC],
                                         w1t_sb[:, g * H:(g + 1) * H], rhs_in,
                                         start=(g == 0), stop=(in_stop and g == 3))
                else:
                    # bias selector first: fills the whole bank
                    nc.tensor.matmul(gb[:, :], b2p_sb[:, :], sel4_sb[:, :],
                                     start=True, stop=False)
                    rhs_in = spk1_sb[:, b * TC:(b + 1) * TC]
                    for g in range(4):
                        nc.tensor.matmul(gb[:, g * TC:(g + 1) * TC],
                                         w2eff_sb[:, g * H:(g + 1) * H], rhs_in,
                                         start=False, stop=(in_stop and g == 3))
                if b >= 2:
                    for g in range(4):
                        nc.tensor.matmul(gb[:, g * TC:(g + 1) * TC],
                                         wspk[:, g * H:(g + 1) * H], sk2,
                                         start=False, stop=False)
                if b >= 1:
                    for g in range(4):
                        nc.tensor.matmul(gb[:, g * TC:(g + 1) * TC],
                                         whh[:, g * H:(g + 1) * H], mp_prev[:, :],
                                         start=False, stop=(g == 3))
                # gate order in bank: g' | i | f | o
                S3 = s3pool.tile([H, 3 * TC], BF, name="S3", tag="S3")
                nc.scalar.activation(S3[:, :], gb[:, 0:3 * TC], AF.Sigmoid)
                So = sopool.tile([H, TC], BF, name="So", tag="So")
                nc.scalar.activation(So[:, :], gb[:, 3 * TC:], AF.Sigmoid)
                # z(b) = u + fs = (i*g)/2 + f*z(b-1)   (z = syn/2)
                fs = vpool.tile([H, TC], BF, name="fs", tag="fs")
                nc.vector.tensor_tensor(fs[:], S3[:, 2 * TC:3 * TC], z_prev[:],
                                        op=OP.mult)
                u = vpool.tile([H, TC], BF, name="u", tag="u")
                nc.vector.scalar_tensor_tensor(
                    u[:], S3[:, 0:TC], 0.5, S3[:, TC:2 * TC],
                    op0=OP.subtract, op1=OP.mult)          # (g'-0.5)*i = i*g/2
                z = zpool.tile([H, TC], BF, name="z", tag="z")
                nc.vector.tensor_tensor(z[:], u[:], fs[:], op=OP.add)
                ts = vpool.tile([H, TC], BF, name="ts", tag="ts")
                nc.scalar.activation(ts[:], z[:], AF.Tanh, scale=2.0)
                mp = mppool.tile([H, TC], BF, name="mp", tag="mp")
                nc.vector.tensor_tensor(mp[:], So[:], ts[:], op=OP.mult)
                # spike: spk(b) = (mp(b) - thr*spk(b-1) - thr > 0)
                #              = (mp - thr) > thr*spk(b-1); thr==1 -> binary in1
                assert thr == 1.0, "general thr needs a scaled spike copy"
                if layer == 1:
                    spk_new = spk1_sb[:, b * TC:(b + 1) * TC]
                    nc.vector.scalar_tensor_tensor(
                        spk_new, mp[:], thr, sk1,
                        op0=OP.subtract, op1=OP.is_gt,
                        accum_out=cntcol[:, b:b + 1])
                else:
                    spk_t = skpool.tile([H, TC], BF, name="spk2", tag="spk2")
                    nc.vector.scalar_tensor_tensor(
                        spk_t[:], mp[:], thr, sk1,
                        op0=OP.subtract, op1=OP.is_gt)
                    spk_new = spk_t[:]
                return z, mp, spk_new

            # ---- phase A: layer-1 scan, record spikes + counts ----
            z_p, mp_p = zeros_sb, zeros_sb
            sk1, sk2 = zeros_sb[:], zeros_sb[:]
            for b in range(STEPS):
                z_p, mp_p, spk_new = lstm_step(b, 1, z_p, mp_p, sk1, sk2, thr1)
                sk2 = sk1
                sk1 = spk_new

            # ---- BN stats: count -> AllReduce -> fold into layer-2 weights ----
            cnt = const.tile([H, 1], F32, name="cnt")
            nc.vector.tensor_reduce(cnt[:], cntcol[:, 0:STEPS],
                                    axis=mybir.AxisListType.X, op=OP.add)
            cc_in = dram.tile([H, 1], F32, name="cc_in")
            cc_out = dram.tile([H, 1], F32, name="cc_out", addr_space="Shared")
            nc.sync.dma_start(cc_in[:], cnt[:])
            nc.gpsimd.collective_compute(
                "AllReduce", OP.add,
                replica_groups=[list(range(NCORES))],
                ins=[cc_in[:]], outs=[cc_out[:]])
            cntg = const.tile([H, 1], F32, name="cntg")
            nc.sync.dma_start(cntg[:], cc_out[:])

            p_t = const.tile([H, 1], F32, name="p_t")
            nc.vector.tensor_scalar(p_t[:], cntg[:], 1.0 / (B * T), None, OP.mult)
            q_t = const.tile([H, 1], F32, name="q_t")
            nc.vector.tensor_scalar(q_t[:], p_t[:], -1.0, 1.0, OP.mult, OP.add)
            var_t = const.tile([H, 1], F32, name="var_t")
            nc.vector.tensor_tensor(var_t[:], p_t[:], q_t[:], op=OP.mult)
            nc.vector.tensor_scalar(var_t[:], var_t[:], EPS, None, OP.add)
            sq_t = const.tile([H, 1], F32, name="sq_t")
            nc.scalar.activation(sq_t[:], var_t[:], AF.Sqrt, bias=0.0)
            rs_t = const.tile([H, 1], F32, name="rs_t")
            nc.vector.reciprocal(rs_t[:], sq_t[:])
            a_t = const.tile([H, 1], F32, name="a_t")
            nc.vector.tensor_tensor(a_t[:], gamma_sb[:], rs_t[:], op=OP.mult)
            pa_t = const.tile([H, 1], F32, name="pa_t")
            nc.vector.tensor_tensor(pa_t[:], p_t[:], a_t[:], op=OP.mult)
            c_t = const.tile([H, 1], F32, name="c_t")
            nc.vector.scalar_tensor_tensor(c_t[:], pa_t[:], -1.0, beta_sb[:],
                                           op0=OP.mult, op1=OP.add)
            cbf_t = const.tile([H, 1], BF, name="cbf_t")
            nc.vector.tensor_copy(cbf_t[:], c_t[:])

            w2eff_sb = const.tile([H, 4 * H], BF, name="w2eff")
            nc.vector.tensor_scalar(w2eff_sb[:], w2t32_sb[:], a_t[:], None, OP.mult)

            bp = fpool.tile([1, 4 * H], F32, name="biasp", tag="biasp")
            nc.tensor.matmul(bp[:, :], cbf_t[:, :], w2tbf_sb[:, :],
                             start=True, stop=True)
            b2eff_sb = const.tile([1, 4 * H], BF, name="b2eff")
            nc.vector.tensor_tensor(b2eff_sb[:], b2sum_sb[:], bp[:, :], op=OP.add)
            # reshape [1,512] -> [4,128] across partitions via linear DRAM
            b2lin = dram.tile([4, H], BF, name="b2lin")
            nc.sync.dma_start(b2lin[:].rearrange("a b -> () (a b)"), b2eff_sb[:])
            b2p_sb = const.tile([4, H], BF, name="b2p")
            nc.sync.dma_start(b2p_sb[:], b2lin[:])

            # ---- phase B: layer-2 scan, fused mean+fc accumulation ----
            # sum_b mem2(b) = sum_b mp(b) - thr2 * sum_{b<STEPS-1} spk2(b)
            fcp = fpool.tile([8, TC], F32, name="fcp", tag="fcp")
            z_p, mp_p = zeros_sb, zeros_sb
            sk1, sk2 = zeros_sb[:], zeros_sb[:]
            for b in range(STEPS):
                z_n, mp_n, spk_new = lstm_step(b, 2, z_p, mp_p, sk1, sk2, thr2)
                if b >= 1:
                    nc.tensor.matmul(fcp[:, :], fcwt_sb[:, :], mp_p[:, :],
                                     start=(b == 1), stop=False)
                    nc.tensor.matmul(fcp[:, :], fcwtn_sb[:, :], sk1,
                                     start=False, stop=False)
                z_p, mp_p = z_n, mp_n
                sk2 = sk1
                sk1 = spk_new
            nc.tensor.matmul(fcp[:, :], fcwt_sb[:, :], mp_p[:, :],
                             start=False, stop=True)

            out_sb = const.tile([8, TC], F32, name="out_sb")
            nc.scalar.activation(out_sb[:], fcp[:, :], AF.Identity,
                                 bias=fcb_sb[:])
            nc.sync.dma_start(out_d[:], out_sb[:])

            if DBG:
                nc.sync.dma_start(spk0_dd[:], spk0_sb[:])
                nc.sync.dma_start(spk1_dd[:], spk1_sb[:])
                nc.sync.dma_start(cnt_dd[:], cnt[:])
                nc.sync.dma_start(b2p_dd[:], b2p_sb[:])
                nc.sync.dma_start(w2e_dd[:], w2eff_sb[:])

    _split_mm_waits(nc)
    return nc


def _split_mm_waits(nc):
    """The S3D3 matmul ISA struct carries only one sync-wait slot; move any
    extra Tile-assigned waits onto a preceding PE NoOp."""
    for fn in nc.m.functions:
        for blk in fn.blocks:
            out = []
            for inst in blk.instructions:
                si = getattr(inst, "sync_info", None)
                if (not isinstance(inst, (mybir.InstEventSemaphore,
                                          mybir.InstAllEngineBarrier))
                        and si is not None and si.on_wait
                        and len(si.on_wait) > 1):
                    # keep the FIRST-listed wait on the instruction (Tile lists
                    # the primary data dep first; it fires last). The remaining
                    # waits ride on SEQ NoOps, which block the sequencer — they
                    # must only carry early-firing sems.
                    for j, w in enumerate(si.on_wait[1:]):
                        nop = mybir.InstNoOp(name=f"{inst.name}-wsplit{j}",
                                             ins=[], outs=[])
                        nop.engine = inst.engine
                        nop.sync_info = mybir.SyncInfo(on_wait=[w],
                                                       on_update=[])
                        out.append(nop)
                    si.on_wait = [si.on_wait[0]]
                out.append(inst)
            blk.instructions[:] = out


def _host_inputs(x, conv_w, conv_b, w_ih1, w_hh1, b_ih1, b_hh1, thr1,
                 w_ih2, w_hh2, b_ih2, b_hh2, thr2, bn_gamma, bn_beta,
                 fc_w, fc_b):
    """Build the per-core input dicts (numpy, host-side)."""
    f32 = np.float32
    # im2col with hi/lo bf16 split per core
    xp = np.pad(np.asarray(x, f32), ((0, 0), (1, 1), (0, 0)))  # [B, T+2, C]
    common = {}
    w3t = np.concatenate([conv_w[:, :, k].T for k in range(3)], axis=0)  # [42,32]
    common["wconv"] = _bf16(np.concatenate(
        [w3t, w3t, np.asarray(conv_b, f32)[None, :]], axis=0))
    w1t = _reorder_gates_cols(np.asarray(w_ih1, f32).T)        # [32, 512]
    b1 = _reorder_gates_cols((np.asarray(b_ih1) + np.asarray(b_hh1))[None, :])
    common["w1t"] = _bf16(np.concatenate([w1t, b1], axis=0))   # [33, 512]
    whh1t = _reorder_gates_cols(np.asarray(w_hh1, f32).T)
    common["whh1t"] = _bf16(whh1t)
    common["wspk1t"] = _bf16(-thr1 * whh1t)
    w2t = _reorder_gates_cols(np.asarray(w_ih2, f32).T)        # [128, 512]
    common["w2t32"] = np.ascontiguousarray(w2t, f32)
    common["w2tbf"] = _bf16(w2t)
    whh2t = _reorder_gates_cols(np.asarray(w_hh2, f32).T)
    common["whh2t"] = _bf16(whh2t)
    common["wspk2t"] = _bf16(-thr2 * whh2t)
    common["b2sum"] = np.ascontiguousarray(
        _reorder_gates_cols((np.asarray(b_ih2) + np.asarray(b_hh2))[None, :]), f32)
    sel = np.zeros((4, 4 * TC), f32)
    for g in range(4):
        sel[g, g * TC:(g + 1) * TC] = 1.0
    common["sel4"] = _bf16(sel)
    fcwt = np.asarray(fc_w, f32).T / STEPS
    common["fcwt"] = _bf16(fcwt)
    common["fcwtn"] = _bf16(-thr2 * fcwt)
    common["fcb"] = np.ascontiguousarray(np.asarray(fc_b, f32)[:, None], f32)
    common["gamma"] = np.ascontiguousarray(np.asarray(bn_gamma, f32)[:, None], f32)
    common["beta"] = np.ascontiguousarray(np.asarray(bn_beta, f32)[:, None], f32)

    in_maps = []
    for k in range(NCORES):
        xw = xp[:, 64 * k: 64 * k + 66, :]                     # [B, 66, C]
        taps = [xw[:, kk:kk + 64, :].transpose(2, 0, 1).reshape(C, B * TC)
                for kk in range(3)]                            # 3 x [14, B*64]
        arr = np.concatenate(taps, axis=0)                     # [42, B*64]
        hi = arr.astype(ml_dtypes.bfloat16)
        lo = (arr - hi.astype(f32)).astype(ml_dtypes.bfloat16)
        ones = np.ones((1, B * TC), ml_dtypes.bfloat16)
        m = dict(common)
        m["xt3"] = np.ascontiguousarray(np.concatenate(
            [hi, lo, ones], axis=0))                           # [85, B*64]
        in_maps.append(m)
    return in_maps


_CACHE = {}


def kernel(x, conv_w, conv_b, w_ih1, w_hh1, b_ih1, b_hh1, thr1,
           w_ih2, w_hh2, b_ih2, b_hh2, thr2, bn_gamma, bn_beta,
           fc_w, fc_b):
    thr1 = float(np.asarray(thr1)); thr2 = float(np.asarray(thr2))
    key = (thr1, thr2)
    if key not in _CACHE:
        _CACHE[key] = build_kernel(thr1, thr2)
    nc = _CACHE[key]
    in_maps = _host_inputs(x, conv_w, conv_b, w_ih1, w_hh1, b_ih1, b_hh1, thr1,
                           w_ih2, w_hh2, b_ih2, b_hh2, thr2, bn_gamma, bn_beta,
                           fc_w, fc_b)
    res = run_bass_kernel_spmd(nc, in_maps, core_ids=list(range(NCORES)),
                               trace=bool(int(os.environ.get("SLSTM_TRACE", "0"))))
    outT = np.concatenate([r["out"] for r in res.results], axis=1)  # [8, 512]
    if res.exec_time_ns is not None:
        kernel.last_exec_time_ns = res.exec_time_ns
    return np.ascontiguousarray(outT.T.astype(np.float32))
